# revision 1
# baseline (speedup 1.0000x reference)
"""Trainium2 Bass kernel for nn_ContinuousCRF (mean-field CRF, 96x96 image, 3 classes).

Key algebraic identity: the dense N^2 pairwise matrix (N=9216) is pure
geometry -- pairwise[n,m] = exp(-dist(n,m)) * (dist<=5), diag=0 -- so
`messages = pairwise @ q` is exactly an 11x11 spatial convolution with 80
nonzero taps.  We implement that conv as 11 accumulating TensorE matmuls
(one banded [96,96] matrix per row offset dy, contracting over the x axis),
plus a 12th identity matmul that adds the unary term into the same PSUM
accumulation.  Softmax over the 3 classes runs on ACT (exp) + DVE
(reduce/recip/mul).

Layout on chip: partitions = x (96), free dims = (c=3, y=96); the q tile is
y-padded by 5 on both sides so all 11 shifted matmul reads are in-bounds.

Sharding: the whole problem is ~40us of work dominated by per-instruction
overheads, so cross-core communication would cost more than it saves; every
core runs the identical full-image program (SPMD replication) and the host
takes core 0's output.
"""

import numpy as np

H = 96
W = 96
C = 3
RAD = 5            # connectivity radius (dist <= 5)
NUM_ITERS = 5
NDY = 2 * RAD + 1  # 11
YPAD = W + 2 * RAD  # padded y extent = 106
N_CORES = 8

_cache = {}


def _kernel_matrix():
    """K[dy+5, dx+5] = exp(-sqrt(dy^2+dx^2)) if 0 < dy^2+dx^2 <= 25 else 0."""
    k = np.zeros((NDY, NDY), np.float64)
    for dy in range(-RAD, RAD + 1):
        for dx in range(-RAD, RAD + 1):
            d2 = dy * dy + dx * dx
            if 0 < d2 <= RAD * RAD:
                k[dy + RAD, dx + RAD] = np.exp(-np.sqrt(float(d2)))
    return k


def _band_matrices():
    """band[x, j, x'] = K[j, x - x' + 5]: out[x'] = sum_x band[x,j,x'] q[x]."""
    k = _kernel_matrix()
    idx = np.arange(H)
    d = idx[:, None] - idx[None, :]          # x - x'
    band = np.zeros((H, NDY, H), np.float32)
    for j in range(NDY):
        vals = np.where(np.abs(d) <= RAD, k[j, np.clip(d + RAD, 0, NDY - 1)], 0.0)
        band[:, j, :] = vals.astype(np.float32)
    return band


def _build_nc(comp):
    """Build and compile the Bass module. comp values are baked as immediates."""
    import concourse.bacc as bacc
    import concourse.tile as tile
    from concourse import mybir

    f32 = mybir.dt.float32
    nc = bacc.Bacc("TRN2", target_bir_lowering=False, debug=False)

    unary_d = nc.dram_tensor("unary_t", [H, C, W], f32, kind="ExternalInput")
    band_d = nc.dram_tensor("band", [H, NDY, H], f32, kind="ExternalInput")
    ident_d = nc.dram_tensor("ident", [H, H], f32, kind="ExternalInput")
    qout_d = nc.dram_tensor("qout", [H, C, W], f32, kind="ExternalOutput")

    Exp = mybir.ActivationFunctionType.Exp
    Add = mybir.AluOpType.add
    Mult = mybir.AluOpType.mult
    AX = mybir.AxisListType.X

    def softmax_stage(work, logits_ap, logits_from_psum, q_writes):
        """exp -> sum_c -> recip -> per-channel scaled writes.

        q_writes: list of (out_ap_fn, coeffs) where coeffs[d] scales E_d;
        out gets (sum_d coeffs[d] * E_d) * R.
        """
        e = work.tile([H, C, W], f32)
        nc.scalar.activation(out=e[:, :, :], in_=logits_ap, func=Exp)
        s = work.tile([H, W], f32)
        # view E as [x, y, c] so the innermost (X) reduction sums over c
        nc.vector.tensor_reduce(
            out=s[:, :],
            in_=e[:, :, :].rearrange("p c y -> p y c"),
            axis=AX,
            op=Add,
        )
        r = work.tile([H, W], f32)
        nc.vector.reciprocal(out=r[:, :], in_=s[:, :])
        for out_ap, coeffs in q_writes:
            nz = [(d, float(cv)) for d, cv in enumerate(coeffs) if cv != 0.0]
            if not nz:
                nc.vector.memset(out_ap, 0.0)
                continue
            # acc = sum_d coeffs[d]*E_d, then * r
            if len(nz) == 1 and nz[0][1] == 1.0:
                d0 = nz[0][0]
                nc.vector.tensor_mul(out=out_ap, in0=e[:, d0, :], in1=r[:, :])
                continue
            acc = work.tile([H, W], f32)
            d0, c0 = nz[0]
            if c0 == 1.0:
                first_in = e[:, d0, :]
                rest = nz[1:]
                cur = None
            else:
                nc.vector.tensor_scalar_mul(out=acc[:, :], in0=e[:, d0, :], scalar1=c0)
                rest = nz[1:]
                cur = acc
                first_in = None
            if cur is None and rest:
                d1, c1 = rest[0]
                nc.vector.scalar_tensor_tensor(
                    out=acc[:, :], in0=e[:, d1, :], scalar=c1, in1=first_in,
                    op0=Mult, op1=Add,
                )
                cur = acc
                rest = rest[1:]
            elif cur is None:
                nc.vector.tensor_mul(out=out_ap, in0=first_in, in1=r[:, :])
                continue
            for d1, c1 in rest:
                nc.vector.scalar_tensor_tensor(
                    out=acc[:, :], in0=e[:, d1, :], scalar=c1, in1=cur[:, :],
                    op0=Mult, op1=Add,
                )
                cur = acc
            nc.vector.tensor_mul(out=out_ap, in0=cur[:, :], in1=r[:, :])

    with tile.TileContext(nc) as tc:
        with (
            tc.tile_pool(name="const", bufs=1) as const,
            tc.tile_pool(name="work", bufs=2) as work,
            tc.tile_pool(name="psum", bufs=2, space="PSUM") as psum,
        ):
            u = const.tile([H, C, W], f32)
            nc.sync.dma_start(out=u[:, :, :], in_=unary_d[:, :, :])
            b = const.tile([H, NDY, H], f32)
            nc.sync.dma_start(out=b[:, :, :], in_=band_d[:, :, :])
            ident = const.tile([H, H], f32)
            nc.sync.dma_start(out=ident[:, :], in_=ident_d[:, :])

            qa = const.tile([H, C, YPAD], f32, tag="qa")
            qb = const.tile([H, C, YPAD], f32, tag="qb")
            nc.gpsimd.memset(qa[:, :, :], 0.0)
            nc.gpsimd.memset(qb[:, :, :], 0.0)

            # comp mixing: q_next[c] = (sum_d comp[c,d] * E_d) * r
            mix_writes = lambda qt: [
                (qt[:, c, RAD:RAD + W], [comp[c, d] for d in range(C)])
                for c in range(C)
            ]
            # final output: plain q (no comp mixing)
            eye = np.eye(C)

            # q0 = softmax(unary)
            softmax_stage(work, u[:, :, :], False, mix_writes(qa))

            cur, nxt = qa, qb
            for t in range(NUM_ITERS):
                m = psum.tile([H, C, W], f32)
                for j in range(NDY):
                    nc.tensor.matmul(
                        m[:, :, :],
                        b[:, j, :],
                        cur[:, :, j:j + W],
                        start=(j == 0),
                        stop=False,
                    )
                nc.tensor.matmul(
                    m[:, :, :], ident[:, :], u[:, :, :], start=False, stop=True,
                )
                last = t == NUM_ITERS - 1
                if last:
                    out_t = work.tile([H, C, W], f32, tag="outt")
                    writes = [
                        (out_t[:, c, :], [eye[c, d] for d in range(C)])
                        for c in range(C)
                    ]
                    softmax_stage(work, m[:, :, :], True, writes)
                    nc.sync.dma_start(out=qout_d[:, :, :], in_=out_t[:, :, :])
                else:
                    softmax_stage(work, m[:, :, :], True, mix_writes(nxt))
                    cur, nxt = nxt, cur

    nc.compile()
    return nc


def get_nc(comp):
    key = comp.tobytes()
    if key not in _cache:
        _cache[key] = _build_nc(comp)
    return _cache[key]


def make_inputs(unary):
    """Host-side layout prep: unary [1,C,H,W] (c,y,x) -> [x, c, y]."""
    unary_t = np.ascontiguousarray(
        np.transpose(unary[0], (2, 0, 1)).astype(np.float32)
    )
    return {
        "unary_t": unary_t,
        "band": _band_matrices(),
        "ident": np.eye(H, dtype=np.float32),
    }


def kernel(**inputs):
    from concourse.bass_utils import run_bass_kernel_spmd

    unary = np.asarray(inputs["unary"], dtype=np.float32)
    comp = np.asarray(inputs["compatibility"], dtype=np.float32)
    assert unary.shape == (1, C, H, W), unary.shape

    nc = get_nc(comp)
    in_map = make_inputs(unary)
    res = run_bass_kernel_spmd(
        nc, [dict(in_map) for _ in range(N_CORES)], core_ids=list(range(N_CORES)),
    )
    q = res.results[0]["qout"]                    # [x, c, y]
    out = np.transpose(q, (1, 2, 0))[None]        # [1, c, y, x]
    return np.ascontiguousarray(out.astype(np.float32))


# revision 16
# speedup vs baseline: 2.3141x; 2.3141x over previous
"""Trainium2 Bass kernel for nn_ContinuousCRF (mean-field CRF, 96x96 image, 3 classes).

Key algebraic identity: the dense N^2 pairwise matrix (N=9216) is pure
geometry -- pairwise[n,m] = exp(-dist(n,m)) * (dist<=5), diag=0 -- so
`messages = pairwise @ q` is exactly an 11x11 spatial convolution with 80
nonzero taps.  We implement that conv as 11 accumulating TensorE matmuls
(one banded [96,96] matrix per row offset dy, contracting over the x axis),
plus a 12th identity matmul that adds the unary term into the same PSUM
accumulation.  Softmax over the 3 classes runs on ACT (exp) + DVE
(reduce/recip/mul).

Layout on chip: partitions = x (96), free dims = (c=3, y=96); the q tile is
y-padded by 5 on both sides so all 11 shifted matmul reads are in-bounds.

Sharding: the whole problem is ~40us of work dominated by per-instruction
overheads, so cross-core communication would cost more than it saves; every
core runs the identical full-image program (SPMD replication) and the host
takes core 0's output.
"""

import numpy as np

H = 96
W = 96
C = 3
RAD = 5            # connectivity radius (dist <= 5)
NUM_ITERS = 5
NDY = 2 * RAD + 1  # 11
YPAD = W + 2 * RAD  # padded y extent = 106
N_CORES = 8

_cache = {}


def _kernel_matrix():
    """K[dy+5, dx+5] = exp(-sqrt(dy^2+dx^2)) if 0 < dy^2+dx^2 <= 25 else 0."""
    k = np.zeros((NDY, NDY), np.float64)
    for dy in range(-RAD, RAD + 1):
        for dx in range(-RAD, RAD + 1):
            d2 = dy * dy + dx * dx
            if 0 < d2 <= RAD * RAD:
                k[dy + RAD, dx + RAD] = np.exp(-np.sqrt(float(d2)))
    return k


def _band_matrices():
    """band[x, j, x'] = K[j, x - x' + 5]: out[x'] = sum_x band[x,j,x'] q[x]."""
    k = _kernel_matrix()
    idx = np.arange(H)
    d = idx[:, None] - idx[None, :]          # x - x'
    band = np.zeros((H, NDY, H), np.float32)
    for j in range(NDY):
        vals = np.where(np.abs(d) <= RAD, k[j, np.clip(d + RAD, 0, NDY - 1)], 0.0)
        band[:, j, :] = vals.astype(np.float32)
    return band


def _build_nc(comp):
    """Build and compile the Bass module. comp values are baked as immediates."""
    import concourse.bacc as bacc
    import concourse.tile as tile
    from concourse import mybir

    f32 = mybir.dt.float32
    f32r = mybir.dt.float32r
    bf16 = mybir.dt.bfloat16
    nc = bacc.Bacc("TRN2", target_bir_lowering=False, debug=False)

    unary_d = nc.dram_tensor("unary_t", [H, C, W], f32r, kind="ExternalInput")
    band_d = nc.dram_tensor("band", [H, NDY, H], f32r, kind="ExternalInput")
    ident_d = nc.dram_tensor("ident", [H, H], f32r, kind="ExternalInput")
    qout_d = nc.dram_tensor("qout", [H, C, W], f32, kind="ExternalOutput")

    Exp = mybir.ActivationFunctionType.Exp
    Add = mybir.AluOpType.add
    Mult = mybir.AluOpType.mult
    AX = mybir.AxisListType.X

    def softmax_stage(work, logits_ap, logits_from_psum, q_writes):
        """exp -> sum_c -> recip -> per-channel scaled writes.

        q_writes: list of (out_ap_fn, coeffs) where coeffs[d] scales E_d;
        out gets (sum_d coeffs[d] * E_d) * R.
        """
        e = work.tile([H, C, W], f32)
        nc.scalar.activation(out=e[:, :, :], in_=logits_ap, func=Exp)
        s = work.tile([H, W], f32)
        # view E as [x, y, c] so the innermost (X) reduction sums over c
        nc.vector.tensor_reduce(
            out=s[:, :],
            in_=e[:, :, :].rearrange("p c y -> p y c"),
            axis=AX,
            op=Add,
        )
        r = work.tile([H, W], f32)
        nc.vector.reciprocal(out=r[:, :], in_=s[:, :])
        if isinstance(q_writes, tuple):
            # identity-compat fast path: one fused mul with r broadcast over c
            out_full = q_writes[0]
            import concourse.bass as bass
            r_ap = r[:, :]
            r_bc = bass.AP(
                tensor=r_ap.tensor,
                offset=r_ap.offset,
                ap=[list(r_ap.ap[0]), [0, C], list(r_ap.ap[1])],
            )
            nc.vector.tensor_mul(out=out_full, in0=e[:, :, :], in1=r_bc)
            return
        for out_ap, coeffs in q_writes:
            nz = [(d, float(cv)) for d, cv in enumerate(coeffs) if cv != 0.0]
            if not nz:
                nc.vector.memset(out_ap, 0.0)
                continue
            # acc = sum_d coeffs[d]*E_d, then * r
            if len(nz) == 1 and nz[0][1] == 1.0:
                d0 = nz[0][0]
                nc.vector.tensor_mul(out=out_ap, in0=e[:, d0, :], in1=r[:, :])
                continue
            acc = work.tile([H, W], f32)
            d0, c0 = nz[0]
            if c0 == 1.0:
                first_in = e[:, d0, :]
                rest = nz[1:]
                cur = None
            else:
                nc.vector.tensor_scalar_mul(out=acc[:, :], in0=e[:, d0, :], scalar1=c0)
                rest = nz[1:]
                cur = acc
                first_in = None
            if cur is None and rest:
                d1, c1 = rest[0]
                nc.vector.scalar_tensor_tensor(
                    out=acc[:, :], in0=e[:, d1, :], scalar=c1, in1=first_in,
                    op0=Mult, op1=Add,
                )
                cur = acc
                rest = rest[1:]
            elif cur is None:
                nc.vector.tensor_mul(out=out_ap, in0=first_in, in1=r[:, :])
                continue
            for d1, c1 in rest:
                nc.vector.scalar_tensor_tensor(
                    out=acc[:, :], in0=e[:, d1, :], scalar=c1, in1=cur[:, :],
                    op0=Mult, op1=Add,
                )
                cur = acc
            nc.vector.tensor_mul(out=out_ap, in0=cur[:, :], in1=r[:, :])

    with tile.TileContext(nc) as tc:
        with (
            tc.tile_pool(name="const", bufs=1) as const,
            tc.tile_pool(name="work", bufs=2) as work,
            tc.tile_pool(name="psum", bufs=2, space="PSUM") as psum,
        ):
            # fp32r inputs: PE consumes fp32r at 1 cycle/row (vs 4 for fp32).
            # External inputs are taken as pre-rounded; the on-chip producers
            # (the DVE q-writes) round on write via their fp32r out dtype.
            u = const.tile([H, C, W], f32r, tag="u_r")
            nc.sync.dma_start(out=u[:, :, :], in_=unary_d[:, :, :])
            b = const.tile([H, NDY, H], f32r, tag="b_r")
            nc.sync.dma_start(out=b[:, :, :], in_=band_d[:, :, :])
            ident = const.tile([H, H], f32r, tag="i_r")
            nc.sync.dma_start(out=ident[:, :], in_=ident_d[:, :])

            # Trigger the exp table load (+its drain) immediately so the
            # first real softmax doesn't pay the ~2.6us load behind the DMAs.
            warm_act = const.tile([1, 1], f32, tag="warmact")
            nc.vector.memset(warm_act[:, :], 0.0)
            nc.scalar.activation(out=warm_act[:, :], in_=warm_act[:, :], func=Exp)

            # PE warm-up: throwaway matmuls during the head (DMA + first
            # softmax) so the real conv matmuls run at full clock.
            warm_in = const.tile([128, 512], bf16, tag="warm")
            nc.vector.memset(warm_in[:, :], 0.0)
            warm_ps = psum.tile([128, 512], f32, tag="warmps")
            for _ in range(4):
                nc.tensor.matmul(
                    warm_ps[:, :], warm_in[:, :128], warm_in[:, :],
                    start=True, stop=True,
                )

            qa = const.tile([H, C, YPAD], f32r, tag="qa")
            qb = const.tile([H, C, YPAD], f32r, tag="qb")
            nc.gpsimd.memset(qa[:, :, :].bitcast(f32), 0.0)
            nc.gpsimd.memset(qb[:, :, :].bitcast(f32), 0.0)

            # comp mixing: q_next[c] = (sum_d comp[c,d] * E_d) * r
            comp_is_eye = np.allclose(comp, np.eye(C))
            if comp_is_eye:
                mix_writes = lambda qt: (qt[:, :, RAD:RAD + W],)
            else:
                mix_writes = lambda qt: [
                    (qt[:, c, RAD:RAD + W], [comp[c, d] for d in range(C)])
                    for c in range(C)
                ]
            # final output: plain q (no comp mixing)
            eye = np.eye(C)

            # q0 = softmax(unary)
            softmax_stage(work, u[:, :, :], False, mix_writes(qa))

            cur, nxt = qa, qb
            for t in range(NUM_ITERS):
                m = psum.tile([H, C, W], f32)
                # unary-add first: it only depends on u, so the PE can run it
                # during the preceding softmax instead of idling.
                nc.tensor.matmul(
                    m[:, :, :], ident[:, :], u[:, :, :], start=True, stop=False,
                )
                for j in range(NDY):
                    nc.tensor.matmul(
                        m[:, :, :],
                        b[:, j, :],
                        cur[:, :, j:j + W],
                        start=False,
                        stop=(j == NDY - 1),
                    )
                last = t == NUM_ITERS - 1
                if last:
                    out_t = work.tile([H, C, W], f32, tag="outt")
                    softmax_stage(work, m[:, :, :], True, (out_t[:, :, :],))
                    nc.sync.dma_start(out=qout_d[:, :, :], in_=out_t[:, :, :])
                else:
                    softmax_stage(work, m[:, :, :], True, mix_writes(nxt))
                    cur, nxt = nxt, cur

    nc.compile()
    return nc


def get_nc(comp):
    key = comp.tobytes()
    if key not in _cache:
        _cache[key] = _build_nc(comp)
    return _cache[key]


def make_inputs(unary):
    """Host-side layout prep: unary [1,C,H,W] (c,y,x) -> [x, c, y]."""
    unary_t = np.ascontiguousarray(
        np.transpose(unary[0], (2, 0, 1)).astype(np.float32)
    )
    return {
        "unary_t": unary_t,
        "band": _band_matrices(),
        "ident": np.eye(H, dtype=np.float32),
    }


def kernel(**inputs):
    from concourse.bass_utils import run_bass_kernel_spmd

    unary = np.asarray(inputs["unary"], dtype=np.float32)
    comp = np.asarray(inputs["compatibility"], dtype=np.float32)
    assert unary.shape == (1, C, H, W), unary.shape

    nc = get_nc(comp)
    in_map = make_inputs(unary)
    res = run_bass_kernel_spmd(
        nc, [dict(in_map) for _ in range(N_CORES)], core_ids=list(range(N_CORES)),
    )
    q = res.results[0]["qout"]                    # [x, c, y]
    out = np.transpose(q, (1, 2, 0))[None]        # [1, c, y, x]
    return np.ascontiguousarray(out.astype(np.float32))
